# revision 10
# baseline (speedup 1.0000x reference)
"""MoE gate kernel for TRN2: logits = x @ w, top-8 over 64 experts, softmax.

Sharding: x [65536, 1024] split by token across 8 cores (8192 tokens each),
fed pre-transposed so the contraction dim (d) lands on SBUF partitions
without on-device input transposes. w [1024, 64] replicated.

Precision/speed: x is shipped as an exact fp16 hi/lo pair
(x == xh + xl + O(2^-22)), and w is split into fp16 hi/lo on device.
logits = xh@wh + xl@wh + xh@wl runs as three full-speed fp16 matmul
passes (3 cyc/col total) instead of fp32's two half-speed passes
(4 cyc/col), with fp32 PSUM accumulation — product error ~2^-22 keeps
expert selection at the fp32 rounding floor (float32r's 11-bit mantissa
would flip ~1% of tokens' top-8 sets; plain bf16 ~8%).

Arrangement (w-stationary): per 1024-token DMA macro (2KB DMA lines),
two 512-token compute halves, each an accumulating 24-matmul chain with
w chunks [128d, 64e] stationary and x chunks [128d, 512tok] moving ->
PSUM logitsT [64, 512]; logitsT is copied to SBUF and PE-transposed per
128-token sub-tile back to [128tok, 64e] for DVE max8/max_index8 top-8.
Softmax is batched over the 4 sub-tiles (ACT exp + DVE reciprocal).

DMA-issue pressure (~600ns of issuing-engine time per dma_start) is
spread: x loads split sync/gpsimd, batched per-half outputs on scalar.
"""

import sys

sys.path.insert(0, "/opt/trn_rl_repo")

from contextlib import ExitStack

import numpy as np

import concourse.bacc as bacc
import concourse.mybir as mybir
import concourse.tile as tile
from concourse import masks
from concourse.bass_utils import run_bass_kernel_spmd

N_CORES = 8
TOKENS = 65536
D = 1024
E = 64
TOPK = 8
TOK_PER_CORE = TOKENS // N_CORES
DMACRO = 1024  # tokens per DMA macro-tile (2KB fp16 lines)
CMACRO = 512  # tokens per compute macro (PSUM bank = 512 f32)
SUBS = CMACRO // 128
KCH = D // 128  # contraction chunks

F32 = mybir.dt.float32
F16 = mybir.dt.float16
U32 = mybir.dt.uint32


def build_program(tok_per_core=TOK_PER_CORE):
    n_dmacro = tok_per_core // DMACRO
    nc = bacc.Bacc(
        "TRN2", target_bir_lowering=False, debug=False, num_devices=N_CORES
    )
    xh_d = nc.dram_tensor("xh", [D, tok_per_core], F16, kind="ExternalInput").ap()
    xl_d = nc.dram_tensor("xl", [D, tok_per_core], F16, kind="ExternalInput").ap()
    w_d = nc.dram_tensor("w", [D, E], F32, kind="ExternalInput").ap()
    n_cmacro = tok_per_core // CMACRO
    scores_d = nc.dram_tensor(
        "scores", [n_cmacro, 128, SUBS, TOPK], F32, kind="ExternalOutput"
    ).ap()
    experts_d = nc.dram_tensor(
        "experts", [n_cmacro, 128, SUBS, TOPK], U32, kind="ExternalOutput"
    ).ap()

    with tile.TileContext(nc) as tc, ExitStack() as ctx:
        wpool = ctx.enter_context(tc.tile_pool(name="wpool", bufs=1))
        xpool = ctx.enter_context(tc.tile_pool(name="xpool", bufs=4))
        ltpool = ctx.enter_context(tc.tile_pool(name="ltpool", bufs=3))
        ptpool = ctx.enter_context(tc.tile_pool(name="ptpool", bufs=4, space="PSUM"))
        ppool = ctx.enter_context(tc.tile_pool(name="ppool", bufs=4, space="PSUM"))
        spool = ctx.enter_context(tc.tile_pool(name="spool", bufs=4))

        ident = wpool.tile([64, 64], F32)
        masks.make_identity(nc, ident[:])

        # load w and split into fp16 hi/lo on device
        w_t = wpool.tile([128, KCH, E], F32)
        nc.sync.dma_start(out=w_t[:], in_=w_d.rearrange("(k p) e -> p k e", p=128))
        wh = wpool.tile([128, KCH, E], F16)
        nc.vector.tensor_copy(wh[:], w_t[:])
        wl32 = wpool.tile([128, KCH, E], F32)
        nc.vector.tensor_sub(wl32[:], w_t[:], wh[:])
        wl = wpool.tile([128, KCH, E], F16)
        nc.vector.tensor_copy(wl[:], wl32[:])

        for m in range(n_dmacro):
            xh_t = xpool.tile([128, KCH, DMACRO], F16, tag="xh_t")
            xl_t = xpool.tile([128, KCH, DMACRO], F16, tag="xl_t")
            for k in range(KCH):
                eng = nc.sync if k % 2 == 0 else nc.gpsimd
                eng.dma_start(
                    out=xh_t[:, k, :],
                    in_=xh_d[k * 128 : (k + 1) * 128, m * DMACRO : (m + 1) * DMACRO],
                )
                eng2 = nc.gpsimd if k % 2 == 0 else nc.sync
                eng2.dma_start(
                    out=xl_t[:, k, :],
                    in_=xl_d[k * 128 : (k + 1) * 128, m * DMACRO : (m + 1) * DMACRO],
                )
            for h in range(DMACRO // CMACRO):
                c0 = h * CMACRO
                logitsT_ps = ptpool.tile([64, CMACRO], F32)
                chain = (
                    [(wh, xh_t)] * KCH + [(wh, xl_t)] * KCH + [(wl, xh_t)] * KCH
                )
                for i, (wv, xv) in enumerate(chain):
                    k = i % KCH
                    nc.tensor.matmul(
                        logitsT_ps[:],
                        wv[:, k, :],
                        xv[:, k, c0 : c0 + CMACRO],
                        start=(i == 0),
                        stop=(i == len(chain) - 1),
                    )
                logitsT = ltpool.tile([64, CMACRO], F32)
                nc.vector.tensor_copy(logitsT[:], logitsT_ps[:])

                vals4 = spool.tile([128, SUBS, TOPK], F32, tag="vals4")
                idx4 = spool.tile([128, SUBS, TOPK], U32, tag="idx4")
                for s in range(SUBS):
                    logits = ppool.tile([128, E], F32)
                    nc.tensor.transpose(
                        logits[:], logitsT[:, s * 128 : (s + 1) * 128], ident[:]
                    )
                    nc.vector.max(vals4[:, s, :], logits[:])
                    nc.vector.max_index(idx4[:, s, :], vals4[:, s, :], logits[:])

                # batched softmax over the 4 sub-tiles: exp(v - max) / sum
                sh4 = spool.tile([128, SUBS, TOPK], F32, tag="sh4")
                nc.vector.tensor_sub(
                    sh4[:],
                    vals4[:],
                    vals4[:, :, 0:1].broadcast_to((128, SUBS, TOPK)),
                )
                ex4 = spool.tile([128, SUBS, TOPK], F32, tag="ex4")
                nc.scalar.activation(
                    ex4[:], sh4[:], mybir.ActivationFunctionType.Exp
                )
                sums4 = spool.tile([128, SUBS, 1], F32, tag="sums4")
                nc.vector.tensor_reduce(
                    sums4[:, :, 0],
                    ex4[:],
                    mybir.AxisListType.X,
                    mybir.AluOpType.add,
                )
                rs4 = spool.tile([128, SUBS, 1], F32, tag="rs4")
                nc.vector.reciprocal(rs4[:], sums4[:])
                sc4 = spool.tile([128, SUBS, TOPK], F32, tag="sc4")
                nc.vector.tensor_mul(
                    sc4[:], ex4[:], rs4[:].broadcast_to((128, SUBS, TOPK))
                )

                cm = (m * DMACRO + c0) // CMACRO
                nc.scalar.dma_start(out=scores_d[cm], in_=sc4[:])
                nc.scalar.dma_start(out=experts_d[cm], in_=idx4[:])

    nc.compile()
    return nc


_PROGRAM = None


def _get_program():
    global _PROGRAM
    if _PROGRAM is None:
        _PROGRAM = build_program()
    return _PROGRAM


def _make_in_maps(x, weights):
    x = np.asarray(x, dtype=np.float32)
    w = np.asarray(weights, dtype=np.float32)
    maps = []
    for i in range(N_CORES):
        xs = np.ascontiguousarray(x[i * TOK_PER_CORE : (i + 1) * TOK_PER_CORE].T)
        xh = xs.astype(np.float16)
        xl = (xs - xh.astype(np.float32)).astype(np.float16)
        maps.append({"xh": xh, "xl": xl, "w": w})
    return maps


def run(x, weights, trace=False):
    nc = _get_program()
    res = run_bass_kernel_spmd(
        nc, _make_in_maps(x, weights), list(range(N_CORES)), trace=trace
    )
    def unblock(a):
        # [n_cmacro, 128, SUBS, TOPK] -> [tok_per_core, TOPK]
        return np.ascontiguousarray(a.transpose(0, 2, 1, 3)).reshape(-1, TOPK)

    scores = np.concatenate(
        [unblock(res.results[i]["scores"]) for i in range(N_CORES)]
    )
    experts = np.concatenate(
        [unblock(res.results[i]["experts"]).astype(np.int32) for i in range(N_CORES)]
    )
    return (scores, experts), res


def kernel(x, weights):
    out, _ = run(x, weights)
    return out
